# revision 24
# baseline (speedup 1.0000x reference)
"""Trainium2 Bass kernel for the cross-head MultiHeadAttention module.

Reference computation (per batch-row r of x flattened to (N*L, E)):
    q = x @ Wq; k = x @ Wk; v = x @ Wv           (E = 1024, H = 16, D = 64)
    energy[r, i, j] = sum_d q[r,i,d] * k[r,j,d]  (cross-head, per position)
    attn = softmax(energy / 32, axis=j)
    out[r, i, :] = sum_j attn[r,i,j] * v[r,j,:]
    y = out.reshape(R, E) @ Wo + bo

Distribution: data-parallel over rows (N*L = 16384 -> 2048 rows/core x 8).

Per-core design (all big matmuls in bf16 on the PE array):
  *  Everything runs in "transposed" layout (features on partitions, rows on
     the free dim), so the four big projections need no on-device transposes:
     QT = Wq.T-as-lhsT @ XT etc., with XT supplied pre-transposed by the host.
  *  Q/K/V are round-tripped through DRAM to re-read them in head-major
     layouts (flat DRAM access patterns allow arbitrary stride shuffles):
       QHT/KHT[d, r, i] (64 partitions), VHT[32*(r%4)+j, r//4, d].
  *  energy: one tiny PE matmul per row (lhsT = QHT[:,r,:], rhs = KHT[:,r,:])
     writing E[r] = (16i x 16j) into psum[32b+i, 32k+j], b = r%4, k = slot.
     64 rows share one psum bank -> softmax runs batched on whole banks.
  *  softmax: memset-psum + additive column mask (-3e38 on the 16 pad cols),
     max/sub/exp/sum/recip/mul, all on (128 x 512) tiles.
  *  A^T: nc.vector.transpose (independent 32x32 block transposes) turns
     A[32b+i, 32k+j] into AT[32b+j, 32k+i] -- a per-row transpose in bulk,
     leaving each row's A^T as a weight-loadable 16-partition slab.
  *  attn @ v: one PE matmul per row-pair: lhsT = VHT slab (16j x (2 rows,
     64d)), rhs = AT slab (16j x (2 rows, 16i-in-32)), psum out
     [64rr+d, 32rr'+i] -> diagonal rr==rr' extracted by 2 strided copies
     per bank into OFT[64h+d, s, i].
  *  y^T: OFT round-trips through DRAM into OT[(i%2)*64+d, i//2, r]
     ((head-pair, d) on partitions, rows on the free dim), so the final
     projection runs as dense full-128-contraction matmuls:
     yt[e-chunk, r] = sum_c Wo-chunk[(i,d), e].T @ OT[:, c, :], 8 chunks,
     N=512; + bo; DMA out.  Output rows come back in a (rr, bank, b, kk)
     order; the host undoes the permutation for free.
"""

import numpy as np
import ml_dtypes

import concourse.bass as bass
from concourse import bacc
import concourse.tile as tile
from concourse import mybir
from concourse.bass_utils import run_bass_kernel_spmd

F32 = mybir.dt.float32
BF16 = mybir.dt.bfloat16
AF = mybir.ActivationFunctionType
ALU = mybir.AluOpType
AX = mybir.AxisListType

E = 1024
H = 16
D = 64
NCORE = 8
NEG = -3.0e38


def build_nc(R, RC, dbg=False):
    """Per-core kernel program: R rows total, processed in passes of RC."""
    NP = R // RC        # passes
    NB = RC // 64       # energy banks per pass (64 rows each)
    SP = RC // 2        # AV row-pairs per pass
    NAV = SP // 16      # AV psum banks per pass (16 pairs each)

    nc = bacc.Bacc("TRN2", target_bir_lowering=False, debug=False)
    if dbg:
        assert NP == 1
        d_oft = nc.dram_tensor("d_oft", [128, H, SP], F32, kind="ExternalOutput")

    xt = nc.dram_tensor("xt", [E, R], BF16, kind="ExternalInput")
    wq = nc.dram_tensor("wq", [E, E], BF16, kind="ExternalInput")
    wk = nc.dram_tensor("wk", [E, E], BF16, kind="ExternalInput")
    wv = nc.dram_tensor("wv", [E, E], BF16, kind="ExternalInput")
    wo = nc.dram_tensor("wo", [E, E], BF16, kind="ExternalInput")
    bo = nc.dram_tensor("bo", [1, E], F32, kind="ExternalInput")
    yt = nc.dram_tensor("yt", [E, R], F32, kind="ExternalOutput")

    with tile.TileContext(nc) as tc:
        with (
            tc.tile_pool(name="wpool", bufs=1) as wpool,      # persistent weights
            tc.tile_pool(name="xpool", bufs=2) as xpool,      # per-pass xt chunk
            tc.tile_pool(name="spool", bufs=2) as spool,      # q/k/v staging
            tc.tile_pool(name="hpool", bufs=1) as hpool,      # vht
            tc.tile_pool(name="qkpool", bufs=2) as qkpool,    # qht2/kht2
            tc.tile_pool(name="apool", bufs=2) as apool,      # softmax temps
            tc.tile_pool(name="opool", bufs=2) as opool,      # OFT
            tc.tile_pool(name="ypool", bufs=4) as ypool,      # y staging
            tc.tile_pool(name="otpool", bufs=2) as otpool,    # OT halves
            tc.tile_pool(name="dram", bufs=2, space="DRAM") as dpool,
            tc.tile_pool(name="pproj", bufs=2, space="PSUM") as pproj,
            tc.tile_pool(name="pe", bufs=3, space="PSUM") as pe_pool,
            tc.tile_pool(name="pav", bufs=1, space="PSUM") as pav,
            tc.tile_pool(name="pyt", bufs=1, space="PSUM") as pyt,
        ):
            # ---- pass-0 x chunk first: the first projection matmul
            # needs xtc[c0] + wq[c0], so x goes ahead of the weight queue ----
            xtc0 = xpool.tile([128, 8, RC], BF16, tag="xtc")
            for cc in range(8):
                nc.sync.dma_start(
                    xtc0[:, cc, :],
                    xt.rearrange("(c p) r -> p c r", p=128)[:, cc, 0:RC],
                )
            # ---- persistent loads ----
            wq_sb = wpool.tile([128, 8, E], BF16, tag="wq")
            wk_sb = wpool.tile([128, 8, E], BF16, tag="wk")
            wv_sb = wpool.tile([128, 8, E], BF16, tag="wv")
            for cc in range(8):
                nc.sync.dma_start(
                    wq_sb[:, cc, :],
                    wq.rearrange("(c p) e -> p c e", p=128)[:, cc, :],
                )
            for cc in range(8):
                nc.scalar.dma_start(
                    wk_sb[:, cc, :],
                    wk.rearrange("(c p) e -> p c e", p=128)[:, cc, :],
                )
            for cc in range(8):
                nc.scalar.dma_start(
                    wv_sb[:, cc, :],
                    wv.rearrange("(c p) e -> p c e", p=128)[:, cc, :],
                )
            # Wo with rows regrouped (i, d) -> (h2=i%2, d) per head-pair
            # chunk c=i//2, so OT-chunk contractions use all 128 partitions.
            wo2_sb = wpool.tile([128, 8, E], BF16, tag="wo2")
            nc.scalar.dma_start(
                wo2_sb[:], wo.rearrange("(c h d) e -> (h d) c e", c=8, h=2)
            )
            bo_sb = wpool.tile([128, 8], F32, tag="bo")
            nc.sync.dma_start(bo_sb[:], bo.rearrange("o (t p) -> p t o", p=128).squeeze(-1))

            HS = SP // 2

            def yt_mms(p, hv, ot):
                for et in range(8):
                    ytp = pyt.tile(
                        [128, SP], F32, tag=f"ytp{et % 2}",
                        name=f"ytp{et % 2}"
                    )
                    for c in range(8):
                        nc.tensor.matmul(
                            ytp[:],
                            wo2_sb[:, c, et * 128:(et + 1) * 128],
                            ot[:, c, :],
                            start=(c == 0),
                            stop=(c == 7),
                        )
                    ys = ypool.tile([128, SP], F32, tag="ys")
                    eng = nc.vector if et % 2 == 0 else nc.scalar
                    if eng is nc.vector:
                        eng.tensor_scalar(
                            ys[:], ytp[:],
                            bo_sb[:, et:et + 1], None, op0=ALU.add
                        )
                    else:
                        eng.add(ys[:], ytp[:], bo_sb[:, et:et + 1])
                    nc.sync.dma_start(
                        yt.rearrange("(t q) r -> q t r", q=128)[
                            :, et, p * RC + hv * SP:p * RC + (hv + 1) * SP
                        ],
                        ys[:],
                    )

            pending_yt = []
            for p in range(NP):
                r0 = p * RC
                # ---- load x chunk (pass 0's was issued up front) ----
                if p == 0:
                    xtc = xtc0
                else:
                    xtc = xpool.tile([128, 8, RC], BF16, tag="xtc")
                    for cc in range(8):
                        nc.sync.dma_start(
                            xtc[:, cc, :],
                            xt.rearrange("(c p) r -> p c r", p=128)[
                                :, cc, r0:r0 + RC
                            ],
                        )

                # ---- projections + DRAM roundtrip (feature-major scratch,
                # so every DMA keeps >=256B contiguous runs) ----
                stage_of = {}

                def project(name, w_sb):
                    dt = dpool.tile([E, RC], BF16, tag=f"dram_{name}")
                    for eg in range(4):
                        stg = spool.tile([128, 2, RC], BF16, tag="stg")
                        for et2 in range(2):
                            et = eg * 2 + et2
                            ps = pproj.tile([128, RC], F32, tag="proj")
                            for c in range(8):
                                nc.tensor.matmul(
                                    ps[:],
                                    w_sb[:, c, et * 128:(et + 1) * 128],
                                    xtc[:, c, :],
                                    start=(c == 0),
                                    stop=(c == 7),
                                )
                            eng = nc.vector if et % 2 == 0 else nc.scalar
                            if eng is nc.vector:
                                eng.tensor_copy(stg[:, et2, :], ps[:])
                            else:
                                eng.copy(stg[:, et2, :], ps[:])
                        nc.sync.dma_start(
                            dt[:].rearrange("(t q) r -> q t r", q=128)[
                                :, eg * 2:eg * 2 + 2, :
                            ],
                            stg[:],
                        )
                    stage_of[name] = dt
                    return dt

                # ---- projections, each followed immediately by its
                # head-major re-read so the read DMAs aren't queued behind
                # the later projections' staging writes.
                # Pairs are (s, s+SP): pr is the pair half; within a pass
                # s = g*64 + bank*16 + k with band g, psum bank, slot k.
                # qht2[64*pr+d, 16*pr'+i, s]: block-diagonal Q^T per pair
                # (off-diagonal blocks stay zero), so ONE matmul computes
                # both rows' 16x16 energies with a 128-deep contraction.
                qht2 = qkpool.tile([128, 32, SP], BF16, tag="qht2")
                kht2 = qkpool.tile([128, 16, SP], BF16, tag="kht2")
                if p < 2:
                    # zero the off-diagonal blocks once per buffer
                    nc.vector.memset(qht2[0:64, 16:32, :], 0.0)
                    nc.vector.memset(qht2[64:128, 0:16, :], 0.0)
                qv = project("q", wq_sb)[:].rearrange("(i d) r -> d i r", i=H)
                kv = project("k", wk_sb)[:].rearrange("(i d) r -> d i r", i=H)
                vv = project("v", wv_sb)[:].rearrange("(j d) r -> j d r", j=H)
                for pr in range(2):
                    nc.sync.dma_start(
                        qht2[64 * pr:64 * pr + 64, 16 * pr:16 * pr + 16, :],
                        qv[:, :, pr * SP:(pr + 1) * SP],
                    )
                    nc.sync.dma_start(
                        kht2[64 * pr:64 * pr + 64, :, :],
                        kv[:, :, pr * SP:(pr + 1) * SP],
                    )
                # vht3[32*g+j, sg, 64*pr+d] = V^T[d, r(pr, g*64+sg)]:
                # AV weight slabs are [16j x (pr,d)=128] per (pair, slot)
                vhtr3 = hpool.tile([128, 2, D, 64], BF16, tag="vhtr3")
                for g in range(4):
                    for pr in range(2):
                        nc.sync.dma_start(
                            vhtr3[32 * g:32 * g + 16, pr, :, :],
                            vv[:, :, pr * SP + g * 64:pr * SP + (g + 1) * 64],
                        )
                # reorder (pr, d, sg) -> (sg, (pr d)) so AV weight slabs are
                # single-free-dim (matmul weights reject 2-dim column APs)
                vht3 = hpool.tile([128, 64, 2 * D], BF16, tag="vht3")
                vhv = vht3[:].rearrange("p s (pr d) -> p s pr d", pr=2)
                nc.scalar.copy(
                    vhv[:, :, 0, :], vhtr3[:, 0, :, :].rearrange("p d s -> p s d")
                )
                nc.vector.tensor_copy(
                    vhv[:, :, 1, :], vhtr3[:, 1, :, :].rearrange("p d s -> p s d")
                )

                while pending_yt:
                    yt_mms(*pending_yt.pop(0))

                oft = opool.tile([128, H, SP], BF16, tag="oft")

                def energy_bank(bank):
                    # ---- energy matmuls: one per pair, 128 pair-rows into
                    # one psum bank at partitions 32g+16pr+i ----
                    ep = pe_pool.tile([128, 16, 32], F32, tag="ep")
                    nc.vector.memset(ep[:, :, 16:32], NEG)
                    for k in range(16):
                        for g in range(4):
                            s = g * 64 + bank * 16 + k
                            nc.tensor.matmul(
                                ep[32 * g:32 * g + 32, k, 0:16],
                                qht2[:, :, s],
                                kht2[:, :, s],
                                start=True,
                                stop=True,
                                tile_position=(0, 32 * g),
                            )
                    return ep

                def softmax_av(bank, ep):
                    # ---- batched softmax over the bank.  exp straight off
                    # psum: energies are ~N(0, 0.25) so no max-subtract is
                    # needed for fp32 exp ----
                    ex = apool.tile([128, 16, 32], F32, tag="ex")
                    nc.scalar.activation(ex[:], ep[:], AF.Exp)
                    sm = apool.tile([128, 16], F32, tag="sm")
                    nc.vector.reduce_sum(sm[:], ex[:], axis=AX.X)
                    rcp = apool.tile([128, 16], F32, tag="rcp")
                    nc.vector.reciprocal(rcp[:], sm[:])
                    at = apool.tile([128, 16, 32], BF16, tag="at")
                    nc.vector.tensor_tensor(
                        at[:], ex[:],
                        rcp[:, :, None].to_broadcast([128, 16, 32]), ALU.mult
                    )
                    att = apool.tile([128, 512], BF16, tag="att")
                    nc.vector.transpose(att[:], at[:].rearrange("p a b -> p (a b)"))

                    # ---- attn @ v: one matmul per pair, both halves in the
                    # 128-partition output, diagonal extracted into OFT ----
                    for g in range(4):
                        avp = pav.tile([128, 16, 32], F32, tag="avp")
                        for k in range(16):
                            nc.tensor.matmul(
                                avp[:, k, :],
                                vht3[32 * g:32 * g + 16, bank * 16 + k, :],
                                att[32 * g:32 * g + 16, 32 * k:32 * k + 32],
                                start=True,
                                stop=True,
                                tile_position=(32 * g, 0),
                            )
                        sl0 = bank * 64 + g * 16
                        eng = nc.vector if g % 2 == 0 else nc.scalar
                        for pr in range(2):
                            src = avp[64 * pr:64 * pr + 64, :,
                                      16 * pr:16 * pr + 16].rearrange(
                                          "p k i -> p i k")
                            dst = oft[64 * pr:64 * pr + 64, :, sl0:sl0 + 16]
                            if eng is nc.vector:
                                eng.tensor_copy(dst, src)
                            else:
                                eng.copy(dst, src)

                # software pipeline: run energy 2 banks ahead of the
                # softmax/AV chain so DVE latency never stalls the PE queue
                NB2 = RC // 128
                eps = {}
                for bank in range(NB2):
                    eps[bank] = energy_bank(bank)
                    if bank >= 2:
                        softmax_av(bank - 2, eps.pop(bank - 2))
                softmax_av(NB2 - 2, eps.pop(NB2 - 2))
                softmax_av(NB2 - 1, eps.pop(NB2 - 1))

                if dbg:
                    do = apool.tile([128, H, SP], F32, tag="dbgo")
                    nc.vector.tensor_copy(do[:], oft[:])
                    nc.sync.dma_start(d_oft[:], do[:])

                # ---- out-proj: OFT -> DRAM -> OT[(i%2,d), i//2, r], then
                # dense full-128-contraction matmuls.  Runs per sl-half:
                # hv0's matmuls go out now (banks 0-1 are long done); hv1's
                # are deferred until after the NEXT pass's projections so
                # the PE queue never stalls on the O2/OT DMA chain at a
                # pass boundary.
                o2 = dpool.tile([E, RC], BF16, tag="o2")
                for hv in range(2):
                    for rr in range(2):
                        nc.sync.dma_start(
                            o2[:, hv * SP + rr * HS:
                               hv * SP + (rr + 1) * HS].rearrange(
                                "(i d) sl -> d i sl", i=H
                            ),
                            oft[64 * rr:64 * rr + 64, :,
                                hv * HS:(hv + 1) * HS],
                        )
                    ot = otpool.tile([128, 8, SP], BF16, tag="ot")
                    nc.sync.dma_start(
                        ot[:],
                        o2[:, hv * SP:(hv + 1) * SP].rearrange(
                            "(c h d) r -> (h d) c r", c=8, h=2
                        ),
                    )
                    if hv == 0:
                        yt_mms(p, 0, ot)
                    else:
                        pending_yt.append((p, 1, ot))

            while pending_yt:
                yt_mms(*pending_yt.pop(0))

    nc.finalize()
    return nc


def row_perm(R, RC):
    """out_col(r): maps local row r to its column in the yt output."""
    r = np.arange(R)
    p, lr = r // RC, r % RC
    SP, HS = RC // 2, RC // 4
    pr, s = lr // SP, lr % SP
    g, bank, k = s // 64, (s % 64) // 16, s % 16
    sl = bank * 64 + g * 16 + k
    hv, sh = sl // HS, sl % HS
    return p * RC + hv * SP + pr * HS + sh


_CACHE = {}


def _get_nc(R, RC, dbg=False):
    key = (R, RC, dbg)
    if key not in _CACHE:
        _CACHE[key] = build_nc(R, RC, dbg)
    return _CACHE[key]


def run_cores(x2d, Wq, Wk, Wv, Wo, bo_v, R=None, RC=512, cores=None, dbg=False,
              **run_kwargs):
    """x2d: (ROWS, E) fp32.  Returns (ROWS, E) fp32."""
    ROWS = x2d.shape[0]
    if cores is None:
        cores = list(range(NCORE))
    n = len(cores)
    if R is None:
        R = ROWS // n
    assert R * n == ROWS
    nc = _get_nc(R, RC, dbg)

    bf = ml_dtypes.bfloat16
    scale = 1.0 / np.sqrt(np.sqrt(float(E)))  # fold E**-0.5 into both Wq, Wk
    wq_b = (Wq.astype(np.float64) * scale).astype(bf)
    wk_b = (Wk.astype(np.float64) * scale).astype(bf)
    wv_b = Wv.astype(bf)
    wo_b = Wo.astype(bf)
    bo_in = bo_v.reshape(1, E).astype(np.float32)

    in_maps = []
    for ci in range(n):
        xs = x2d[ci * R:(ci + 1) * R].T  # (E, R)
        in_maps.append({
            "xt": np.ascontiguousarray(xs).astype(bf),
            "wq": wq_b, "wk": wk_b, "wv": wv_b, "wo": wo_b, "bo": bo_in,
        })
    res = run_bass_kernel_spmd(nc, in_maps, core_ids=cores, **run_kwargs)
    perm = row_perm(R, RC)
    out = np.empty((ROWS, E), dtype=np.float32)
    for ci in range(n):
        ytd = res.results[ci]["yt"]  # (E, R)
        out[ci * R:(ci + 1) * R] = ytd[:, perm].T
    if dbg:
        return out, res.results
    if run_kwargs.get("trace"):
        return out, res
    return out


def kernel(x, Wq, Wk, Wv, Wo, bo):
    x = np.asarray(x, dtype=np.float32)
    N, L, _ = x.shape
    y = run_cores(
        x.reshape(N * L, E),
        np.asarray(Wq, np.float32), np.asarray(Wk, np.float32),
        np.asarray(Wv, np.float32), np.asarray(Wo, np.float32),
        np.asarray(bo, np.float32),
    )
    return y.reshape(N, L, E)



# revision 25
# speedup vs baseline: 1.0427x; 1.0427x over previous
"""Trainium2 Bass kernel for the cross-head MultiHeadAttention module.

Reference computation (per batch-row r of x flattened to (N*L, E)):
    q = x @ Wq; k = x @ Wk; v = x @ Wv           (E = 1024, H = 16, D = 64)
    energy[r, i, j] = sum_d q[r,i,d] * k[r,j,d]  (cross-head, per position)
    attn = softmax(energy / 32, axis=j)
    out[r, i, :] = sum_j attn[r,i,j] * v[r,j,:]
    y = out.reshape(R, E) @ Wo + bo

Distribution: data-parallel over rows (N*L = 16384 -> 2048 rows/core x 8).

Per-core design (all big matmuls in bf16 on the PE array):
  *  Everything runs in "transposed" layout (features on partitions, rows on
     the free dim), so the four big projections need no on-device transposes:
     QT = Wq.T-as-lhsT @ XT etc., with XT supplied pre-transposed by the host.
  *  Q/K/V are round-tripped through DRAM to re-read them in head-major
     layouts (flat DRAM access patterns allow arbitrary stride shuffles):
       QHT/KHT[d, r, i] (64 partitions), VHT[32*(r%4)+j, r//4, d].
  *  energy: one tiny PE matmul per row (lhsT = QHT[:,r,:], rhs = KHT[:,r,:])
     writing E[r] = (16i x 16j) into psum[32b+i, 32k+j], b = r%4, k = slot.
     64 rows share one psum bank -> softmax runs batched on whole banks.
  *  softmax: memset-psum + additive column mask (-3e38 on the 16 pad cols),
     max/sub/exp/sum/recip/mul, all on (128 x 512) tiles.
  *  A^T: nc.vector.transpose (independent 32x32 block transposes) turns
     A[32b+i, 32k+j] into AT[32b+j, 32k+i] -- a per-row transpose in bulk,
     leaving each row's A^T as a weight-loadable 16-partition slab.
  *  attn @ v: one PE matmul per row-pair: lhsT = VHT slab (16j x (2 rows,
     64d)), rhs = AT slab (16j x (2 rows, 16i-in-32)), psum out
     [64rr+d, 32rr'+i] -> diagonal rr==rr' extracted by 2 strided copies
     per bank into OFT[64h+d, s, i].
  *  y^T: OFT round-trips through DRAM into OT[(i%2)*64+d, i//2, r]
     ((head-pair, d) on partitions, rows on the free dim), so the final
     projection runs as dense full-128-contraction matmuls:
     yt[e-chunk, r] = sum_c Wo-chunk[(i,d), e].T @ OT[:, c, :], 8 chunks,
     N=512; + bo; DMA out.  Output rows come back in a (rr, bank, b, kk)
     order; the host undoes the permutation for free.
"""

import numpy as np
import ml_dtypes

import concourse.bass as bass
from concourse import bacc
import concourse.tile as tile
from concourse import mybir
from concourse.bass_utils import run_bass_kernel_spmd

F32 = mybir.dt.float32
BF16 = mybir.dt.bfloat16
AF = mybir.ActivationFunctionType
ALU = mybir.AluOpType
AX = mybir.AxisListType

E = 1024
H = 16
D = 64
NCORE = 8
NEG = -3.0e38


def build_nc(R, RC, dbg=False):
    """Per-core kernel program: R rows total, processed in passes of RC."""
    NP = R // RC        # passes
    NB = RC // 64       # energy banks per pass (64 rows each)
    SP = RC // 2        # AV row-pairs per pass
    NAV = SP // 16      # AV psum banks per pass (16 pairs each)

    nc = bacc.Bacc("TRN2", target_bir_lowering=False, debug=False)
    if dbg:
        assert NP == 1
        d_oft = nc.dram_tensor("d_oft", [128, H, SP], F32, kind="ExternalOutput")

    xt = nc.dram_tensor("xt", [E, R], BF16, kind="ExternalInput")
    wq = nc.dram_tensor("wq", [E, E], BF16, kind="ExternalInput")
    wk = nc.dram_tensor("wk", [E, E], BF16, kind="ExternalInput")
    wv = nc.dram_tensor("wv", [E, E], BF16, kind="ExternalInput")
    wo = nc.dram_tensor("wo", [E, E], BF16, kind="ExternalInput")
    bo = nc.dram_tensor("bo", [1, E], F32, kind="ExternalInput")
    yt = nc.dram_tensor("yt", [E, R], F32, kind="ExternalOutput")

    with tile.TileContext(nc) as tc:
        with (
            tc.tile_pool(name="wpool", bufs=1) as wpool,      # persistent weights
            tc.tile_pool(name="xpool", bufs=2) as xpool,      # per-pass xt chunk
            tc.tile_pool(name="spool", bufs=2) as spool,      # q/k/v staging
            tc.tile_pool(name="hpool", bufs=1) as hpool,      # vht
            tc.tile_pool(name="qkpool", bufs=2) as qkpool,    # qht2/kht2
            tc.tile_pool(name="apool", bufs=2) as apool,      # softmax temps
            tc.tile_pool(name="opool", bufs=2) as opool,      # OFT
            tc.tile_pool(name="ypool", bufs=2) as ypool,      # y staging
            tc.tile_pool(name="otpool", bufs=2) as otpool,    # OT halves
            tc.tile_pool(name="dram", bufs=2, space="DRAM") as dpool,
            tc.tile_pool(name="pproj", bufs=2, space="PSUM") as pproj,
            tc.tile_pool(name="pe", bufs=3, space="PSUM") as pe_pool,
            tc.tile_pool(name="pav", bufs=1, space="PSUM") as pav,
            tc.tile_pool(name="pyt", bufs=1, space="PSUM") as pyt,
        ):
            # ---- pass-0 x chunk first: the first projection matmul
            # needs xtc[c0] + wq[c0], so x goes ahead of the weight queue ----
            xtc0 = xpool.tile([128, 8, RC], BF16, tag="xtc")
            for cc in range(8):
                nc.sync.dma_start(
                    xtc0[:, cc, :],
                    xt.rearrange("(c p) r -> p c r", p=128)[:, cc, 0:RC],
                )
            # ---- persistent loads ----
            wq_sb = wpool.tile([128, 8, E], BF16, tag="wq")
            wk_sb = wpool.tile([128, 8, E], BF16, tag="wk")
            wv_sb = wpool.tile([128, 8, E], BF16, tag="wv")
            for cc in range(8):
                nc.sync.dma_start(
                    wq_sb[:, cc, :],
                    wq.rearrange("(c p) e -> p c e", p=128)[:, cc, :],
                )
            for cc in range(8):
                nc.scalar.dma_start(
                    wk_sb[:, cc, :],
                    wk.rearrange("(c p) e -> p c e", p=128)[:, cc, :],
                )
            for cc in range(8):
                nc.scalar.dma_start(
                    wv_sb[:, cc, :],
                    wv.rearrange("(c p) e -> p c e", p=128)[:, cc, :],
                )
            # Wo with rows regrouped (i, d) -> (h2=i%2, d) per head-pair
            # chunk c=i//2, so OT-chunk contractions use all 128 partitions.
            wo2_sb = wpool.tile([128, 8, E], BF16, tag="wo2")
            nc.scalar.dma_start(
                wo2_sb[:], wo.rearrange("(c h d) e -> (h d) c e", c=8, h=2)
            )
            bo_sb = wpool.tile([128, 8], F32, tag="bo")
            nc.sync.dma_start(bo_sb[:], bo.rearrange("o (t p) -> p t o", p=128).squeeze(-1))

            HS = SP // 2

            def yt_mms(p, hv, ot):
                for et in range(8):
                    ytp = pyt.tile(
                        [128, SP], F32, tag=f"ytp{et % 2}",
                        name=f"ytp{et % 2}"
                    )
                    for c in range(8):
                        nc.tensor.matmul(
                            ytp[:],
                            wo2_sb[:, c, et * 128:(et + 1) * 128],
                            ot[:, c, :],
                            start=(c == 0),
                            stop=(c == 7),
                        )
                    ys = ypool.tile([128, SP], F32, tag="ys")
                    eng = nc.vector if et % 2 == 0 else nc.scalar
                    if eng is nc.vector:
                        eng.tensor_scalar(
                            ys[:], ytp[:],
                            bo_sb[:, et:et + 1], None, op0=ALU.add
                        )
                    else:
                        eng.add(ys[:], ytp[:], bo_sb[:, et:et + 1])
                    nc.sync.dma_start(
                        yt.rearrange("(t q) r -> q t r", q=128)[
                            :, et, p * RC + hv * SP:p * RC + (hv + 1) * SP
                        ],
                        ys[:],
                    )

            pending_yt = []
            for p in range(NP):
                r0 = p * RC
                # ---- load x chunk (pass 0's was issued up front) ----
                if p == 0:
                    xtc = xtc0
                else:
                    xtc = xpool.tile([128, 8, RC], BF16, tag="xtc")
                    for cc in range(8):
                        nc.sync.dma_start(
                            xtc[:, cc, :],
                            xt.rearrange("(c p) r -> p c r", p=128)[
                                :, cc, r0:r0 + RC
                            ],
                        )

                # ---- projections + DRAM roundtrip (feature-major scratch,
                # so every DMA keeps >=256B contiguous runs) ----
                stage_of = {}

                def project(name, w_sb):
                    dt = dpool.tile([E, RC], BF16, tag=f"dram_{name}")
                    for eg in range(4):
                        stg = spool.tile([128, 2, RC], BF16, tag="stg")
                        for et2 in range(2):
                            et = eg * 2 + et2
                            ps = pproj.tile([128, RC], F32, tag="proj")
                            for c in range(8):
                                nc.tensor.matmul(
                                    ps[:],
                                    w_sb[:, c, et * 128:(et + 1) * 128],
                                    xtc[:, c, :],
                                    start=(c == 0),
                                    stop=(c == 7),
                                )
                            eng = nc.vector if et % 2 == 0 else nc.scalar
                            if eng is nc.vector:
                                eng.tensor_copy(stg[:, et2, :], ps[:])
                            else:
                                eng.copy(stg[:, et2, :], ps[:])
                        nc.sync.dma_start(
                            dt[:].rearrange("(t q) r -> q t r", q=128)[
                                :, eg * 2:eg * 2 + 2, :
                            ],
                            stg[:],
                        )
                    stage_of[name] = dt
                    return dt

                # ---- projections, each followed immediately by its
                # head-major re-read so the read DMAs aren't queued behind
                # the later projections' staging writes.
                # Pairs are (s, s+SP): pr is the pair half; within a pass
                # s = g*64 + bank*16 + k with band g, psum bank, slot k.
                # qht2[64*pr+d, 16*pr'+i, s]: block-diagonal Q^T per pair
                # (off-diagonal blocks stay zero), so ONE matmul computes
                # both rows' 16x16 energies with a 128-deep contraction.
                qht2 = qkpool.tile([128, 32, SP], BF16, tag="qht2")
                kht2 = qkpool.tile([128, 16, SP], BF16, tag="kht2")
                if p < 2:
                    # zero the off-diagonal blocks once per buffer
                    nc.vector.memset(qht2[0:64, 16:32, :], 0.0)
                    nc.vector.memset(qht2[64:128, 0:16, :], 0.0)
                qv = project("q", wq_sb)[:].rearrange("(i d) r -> d i r", i=H)
                kv = project("k", wk_sb)[:].rearrange("(i d) r -> d i r", i=H)
                vv = project("v", wv_sb)[:].rearrange("(j d) r -> j d r", j=H)
                for pr in range(2):
                    nc.sync.dma_start(
                        qht2[64 * pr:64 * pr + 64, 16 * pr:16 * pr + 16, :],
                        qv[:, :, pr * SP:(pr + 1) * SP],
                    )
                    nc.sync.dma_start(
                        kht2[64 * pr:64 * pr + 64, :, :],
                        kv[:, :, pr * SP:(pr + 1) * SP],
                    )
                # vht3[32*g+j, sg, 64*pr+d] = V^T[d, r(pr, g*64+sg)]:
                # AV weight slabs are [16j x (pr,d)=128] per (pair, slot)
                vhtr3 = hpool.tile([128, 2, D, 64], BF16, tag="vhtr3")
                for g in range(4):
                    for pr in range(2):
                        nc.sync.dma_start(
                            vhtr3[32 * g:32 * g + 16, pr, :, :],
                            vv[:, :, pr * SP + g * 64:pr * SP + (g + 1) * 64],
                        )
                # reorder (pr, d, sg) -> (sg, (pr d)) so AV weight slabs are
                # single-free-dim (matmul weights reject 2-dim column APs)
                vht3 = hpool.tile([128, 64, 2 * D], BF16, tag="vht3")
                vhv = vht3[:].rearrange("p s (pr d) -> p s pr d", pr=2)
                nc.scalar.copy(
                    vhv[:, :, 0, :], vhtr3[:, 0, :, :].rearrange("p d s -> p s d")
                )
                nc.vector.tensor_copy(
                    vhv[:, :, 1, :], vhtr3[:, 1, :, :].rearrange("p d s -> p s d")
                )

                while pending_yt:
                    yt_mms(*pending_yt.pop(0))

                oft = opool.tile([128, H, SP], BF16, tag="oft")

                def energy_bank(bank):
                    # ---- energy matmuls: one per pair, 128 pair-rows into
                    # one psum bank at partitions 32g+16pr+i ----
                    ep = pe_pool.tile([128, 16, 32], F32, tag="ep")
                    nc.vector.memset(ep[:, :, 16:32], NEG)
                    for k in range(16):
                        for g in range(4):
                            s = g * 64 + bank * 16 + k
                            nc.tensor.matmul(
                                ep[32 * g:32 * g + 32, k, 0:16],
                                qht2[:, :, s],
                                kht2[:, :, s],
                                start=True,
                                stop=True,
                                tile_position=(0, 32 * g),
                            )
                    return ep

                def softmax_av(bank, ep):
                    # ---- batched softmax over the bank.  exp straight off
                    # psum: energies are ~N(0, 0.25) so no max-subtract is
                    # needed for fp32 exp ----
                    ex = apool.tile([128, 16, 32], F32, tag="ex")
                    nc.scalar.activation(ex[:], ep[:], AF.Exp)
                    sm = apool.tile([128, 16], F32, tag="sm")
                    nc.vector.reduce_sum(sm[:], ex[:], axis=AX.X)
                    rcp = apool.tile([128, 16], F32, tag="rcp")
                    nc.vector.reciprocal(rcp[:], sm[:])
                    at = apool.tile([128, 16, 32], BF16, tag="at")
                    nc.vector.tensor_tensor(
                        at[:], ex[:],
                        rcp[:, :, None].to_broadcast([128, 16, 32]), ALU.mult
                    )
                    att = apool.tile([128, 512], BF16, tag="att")
                    nc.vector.transpose(att[:], at[:].rearrange("p a b -> p (a b)"))

                    # ---- attn @ v: one matmul per pair, both halves in the
                    # 128-partition output, diagonal extracted into OFT ----
                    for g in range(4):
                        avp = pav.tile([128, 16, 32], F32, tag="avp")
                        for k in range(16):
                            nc.tensor.matmul(
                                avp[:, k, :],
                                vht3[32 * g:32 * g + 16, bank * 16 + k, :],
                                att[32 * g:32 * g + 16, 32 * k:32 * k + 32],
                                start=True,
                                stop=True,
                                tile_position=(32 * g, 0),
                            )
                        sl0 = bank * 64 + g * 16
                        eng = nc.vector if g % 2 == 0 else nc.scalar
                        for pr in range(2):
                            src = avp[64 * pr:64 * pr + 64, :,
                                      16 * pr:16 * pr + 16].rearrange(
                                          "p k i -> p i k")
                            dst = oft[64 * pr:64 * pr + 64, :, sl0:sl0 + 16]
                            if eng is nc.vector:
                                eng.tensor_copy(dst, src)
                            else:
                                eng.copy(dst, src)

                # software pipeline: run energy 2 banks ahead of the
                # softmax/AV chain so DVE latency never stalls the PE queue
                NB2 = RC // 128
                eps = {}
                for bank in range(NB2):
                    eps[bank] = energy_bank(bank)
                    if bank >= 2:
                        softmax_av(bank - 2, eps.pop(bank - 2))
                softmax_av(NB2 - 2, eps.pop(NB2 - 2))
                softmax_av(NB2 - 1, eps.pop(NB2 - 1))

                if dbg:
                    do = apool.tile([128, H, SP], F32, tag="dbgo")
                    nc.vector.tensor_copy(do[:], oft[:])
                    nc.sync.dma_start(d_oft[:], do[:])

                # ---- out-proj: OFT -> DRAM -> OT[(i%2,d), i//2, r], then
                # dense full-128-contraction matmuls.  Runs per sl-half:
                # hv0's matmuls go out now (banks 0-1 are long done); hv1's
                # are deferred until after the NEXT pass's projections so
                # the PE queue never stalls on the O2/OT DMA chain at a
                # pass boundary.
                o2 = dpool.tile([E, RC], BF16, tag="o2")
                for hv in range(2):
                    for rr in range(2):
                        nc.scalar.dma_start(
                            o2[:, hv * SP + rr * HS:
                               hv * SP + (rr + 1) * HS].rearrange(
                                "(i d) sl -> d i sl", i=H
                            ),
                            oft[64 * rr:64 * rr + 64, :,
                                hv * HS:(hv + 1) * HS],
                        )
                    ot = otpool.tile([128, 8, SP], BF16, tag="ot")
                    nc.scalar.dma_start(
                        ot[:],
                        o2[:, hv * SP:(hv + 1) * SP].rearrange(
                            "(c h d) r -> (h d) c r", c=8, h=2
                        ),
                    )
                    if hv == 0:
                        yt_mms(p, 0, ot)
                    else:
                        pending_yt.append((p, 1, ot))

            while pending_yt:
                yt_mms(*pending_yt.pop(0))

    nc.finalize()
    return nc


def row_perm(R, RC):
    """out_col(r): maps local row r to its column in the yt output."""
    r = np.arange(R)
    p, lr = r // RC, r % RC
    SP, HS = RC // 2, RC // 4
    pr, s = lr // SP, lr % SP
    g, bank, k = s // 64, (s % 64) // 16, s % 16
    sl = bank * 64 + g * 16 + k
    hv, sh = sl // HS, sl % HS
    return p * RC + hv * SP + pr * HS + sh


_CACHE = {}


def _get_nc(R, RC, dbg=False):
    key = (R, RC, dbg)
    if key not in _CACHE:
        _CACHE[key] = build_nc(R, RC, dbg)
    return _CACHE[key]


def run_cores(x2d, Wq, Wk, Wv, Wo, bo_v, R=None, RC=512, cores=None, dbg=False,
              **run_kwargs):
    """x2d: (ROWS, E) fp32.  Returns (ROWS, E) fp32."""
    ROWS = x2d.shape[0]
    if cores is None:
        cores = list(range(NCORE))
    n = len(cores)
    if R is None:
        R = ROWS // n
    assert R * n == ROWS
    nc = _get_nc(R, RC, dbg)

    bf = ml_dtypes.bfloat16
    scale = 1.0 / np.sqrt(np.sqrt(float(E)))  # fold E**-0.5 into both Wq, Wk
    wq_b = (Wq.astype(np.float64) * scale).astype(bf)
    wk_b = (Wk.astype(np.float64) * scale).astype(bf)
    wv_b = Wv.astype(bf)
    wo_b = Wo.astype(bf)
    bo_in = bo_v.reshape(1, E).astype(np.float32)

    in_maps = []
    for ci in range(n):
        xs = x2d[ci * R:(ci + 1) * R].T  # (E, R)
        in_maps.append({
            "xt": np.ascontiguousarray(xs).astype(bf),
            "wq": wq_b, "wk": wk_b, "wv": wv_b, "wo": wo_b, "bo": bo_in,
        })
    res = run_bass_kernel_spmd(nc, in_maps, core_ids=cores, **run_kwargs)
    perm = row_perm(R, RC)
    out = np.empty((ROWS, E), dtype=np.float32)
    for ci in range(n):
        ytd = res.results[ci]["yt"]  # (E, R)
        out[ci * R:(ci + 1) * R] = ytd[:, perm].T
    if dbg:
        return out, res.results
    if run_kwargs.get("trace"):
        return out, res
    return out


def kernel(x, Wq, Wk, Wv, Wo, bo):
    x = np.asarray(x, dtype=np.float32)
    N, L, _ = x.shape
    y = run_cores(
        x.reshape(N * L, E),
        np.asarray(Wq, np.float32), np.asarray(Wk, np.float32),
        np.asarray(Wv, np.float32), np.asarray(Wo, np.float32),
        np.asarray(bo, np.float32),
    )
    return y.reshape(N, L, E)

